# revision 2
# baseline (speedup 1.0000x reference)
"""Trainium2 Bass kernel v2 for NRI-style GNN decoder (nn_Decoder_58600533787128).

Data-parallel over batch across 8 NeuronCores, bf16 matmul datapath.

v2 changes over v1:
  - b1 folded into the A/B node-feature GEMM via a ones-row (K=64 -> 65);
    h1 eviction is a pure relu (no bias operand).
  - b2 broadcast matmul eliminated: layer-2 eviction computes
    max(z, -b2) (DVE tensor_tensor against a resident -b2 tile) and the
    +b2 compensation  sum_t b2_t (x) rowsum_i(rel_type_t)  is computed on
    host per batch and added (fp32) when agg PSUM is copied into augT.
  - PSUM repack: ab shares the mp pool, agg double-buffered.
  - layer-2 matmuls interleaved between the long h1 matmuls (LDW hiding).
  - ACT/DVE eviction rotation rebalanced (msc eviction is DVE-only now).
"""
import sys

sys.path.insert(0, "/opt/trn_rl_repo")

import numpy as np
import ml_dtypes

BF16 = ml_dtypes.bfloat16

B, N, F, H, O, T, E = 64, 64, 64, 256, 64, 4, 4032
EP = 4096           # padded edge count (full N*N grid, diagonal dead)
NT = EP // 128      # 32 edge tiles of 128 (2 receiver blocks each)
NCORES = 8
BPC = B // NCORES   # batches per core
NST2 = 4            # 4 supertiles of 1024 edges per batch


def build_nc(bpc=BPC, num_devices=NCORES, reps=1):
    import concourse.mybir as mybir
    from concourse import bacc, tile

    dtf = mybir.dt.float32
    dtb = mybir.dt.bfloat16
    AF = mybir.ActivationFunctionType
    ALU = mybir.AluOpType

    nc = bacc.Bacc(
        "TRN2", target_bir_lowering=False, debug=False, num_devices=num_devices
    )
    xT_d = nc.declare_dram_parameter("xT", [bpc, F + 1, N], dtb, isOutput=False)
    rtm_d = nc.declare_dram_parameter("rtm", [bpc, 128, T * NT * 2], dtb, isOutput=False)
    corr_d = nc.declare_dram_parameter("corr", [bpc, O, N], dtf, isOutput=False)
    srp_d = nc.declare_dram_parameter("srp", [128, EP], dtb, isOutput=False)
    w1_d = nc.declare_dram_parameter("w1s", [F + 1, 2 * T * H], dtb, isOutput=False)
    w2_d = nc.declare_dram_parameter("w2s", [128, T * 2 * O], dtb, isOutput=False)
    nb2_d = nc.declare_dram_parameter("nb2", [128, T * 512], dtb, isOutput=False)
    ow1_d = nc.declare_dram_parameter("ow1s", [128, H], dtb, isOutput=False)
    ow2_d = nc.declare_dram_parameter("ow2s", [128, 4 * 128], dtb, isOutput=False)
    ow3_d = nc.declare_dram_parameter("ow3s", [128, 2 * O], dtb, isOutput=False)
    ob1_d = nc.declare_dram_parameter("ob1c", [128, 2], dtf, isOutput=False)
    ob2_d = nc.declare_dram_parameter("ob2c", [128, 2], dtf, isOutput=False)
    ob3_d = nc.declare_dram_parameter("ob3c", [O, 1], dtf, isOutput=False)
    y_d = nc.declare_dram_parameter("y", [O, bpc * N], dtf, isOutput=True)

    NK = bpc * 4 * NST2  # pipeline stages: (batch, type, st2)

    with tile.TileContext(nc) as tc:
        with (
            tc.tile_pool(name="const", bufs=1) as cpool,
            tc.tile_pool(name="work", bufs=4) as wpool,
            tc.tile_pool(name="absb", bufs=2) as abpool_sb,
            tc.tile_pool(name="h1sb", bufs=6) as h1pool,
            tc.tile_pool(name="mscsb", bufs=6) as mscpool,
            tc.tile_pool(name="hps", bufs=2, space="PSUM") as hpsum,
            tc.tile_pool(name="mps", bufs=2, space="PSUM") as mpsum,
            tc.tile_pool(name="aggps", bufs=2, space="PSUM") as apsum,
        ):
            # resident constants (one DMA each; layouts prepped host-side)
            srp = cpool.tile([128, EP], dtb)
            nc.sync.dma_start(srp[:], srp_d[:])
            w1s = cpool.tile([F + 1, 2 * T * H], dtb)
            nc.sync.dma_start(w1s[:], w1_d[:])
            w2s = cpool.tile([128, T * 2 * O], dtb)
            nc.sync.dma_start(w2s[:], w2_d[:])
            nb2 = cpool.tile([128, T * 512], dtb)
            nc.sync.dma_start(nb2[:], nb2_d[:])
            ow1s = cpool.tile([128, H], dtb)
            nc.sync.dma_start(ow1s[:], ow1_d[:])
            ow2s = cpool.tile([128, 4 * 128], dtb)
            nc.sync.dma_start(ow2s[:], ow2_d[:])
            ow3s = cpool.tile([128, 2 * O], dtb)
            nc.sync.dma_start(ow3s[:], ow3_d[:])
            ob1c = cpool.tile([128, 2], dtf)
            nc.sync.dma_start(ob1c[:], ob1_d[:])
            ob2c = cpool.tile([128, 2], dtf)
            nc.sync.dma_start(ob2c[:], ob2_d[:])
            ob3c = cpool.tile([O, 1], dtf)
            nc.sync.dma_start(ob3c[:], ob3_d[:])

            import contextlib
            augT = wpool.tile([128, bpc * N], dtb, tag="augT")
            if True:
                xt_t = {}
                rtm_t = {}
                corr_t = {}
                ab_t = {}
                h1_t = {}    # (k, c) -> sbuf tile [128, 1024]
                msc_t = {}   # k -> sbuf tile [128, 512]
                agg_t = {}   # b -> psum tile [64, 64]
                ew_ctr = [0]

                def emit_dma(b):
                    xt_t[b] = wpool.tile([F + 1, N], dtb, tag="xt", name="xt")
                    nc.sync.dma_start(xt_t[b][:], xT_d[b])
                    rtm_t[b] = wpool.tile([128, T * NT * 2], dtb, tag="rtm", name="rtm")
                    nc.sync.dma_start(rtm_t[b][:], rtm_d[b])
                    corr_t[b] = wpool.tile([O, N], dtf, tag="corr", name="corr")
                    nc.sync.dma_start(corr_t[b][:], corr_d[b])

                def emit_ab(b, half):
                    # A/B node features for 2 edge types: [A_t|A_t+1 ; B_t|B_t+1]
                    # (b1/2 folded in via the ones-row of xT')
                    if half == 0:
                        ab_t[b] = abpool_sb.tile([128, T * 256], dtb, tag="abs", name="abs")
                    abp = mpsum.tile([128, 512], dtf, tag="mp", name="abp")
                    nc.tensor.matmul(
                        abp[0:64, :], xt_t[b][:], w1s[:, half * 512:(half + 1) * 512],
                        start=True, stop=True, skip_group_check=True,
                    )
                    nc.tensor.matmul(
                        abp[64:128, :], xt_t[b][:],
                        w1s[:, T * H + half * 512: T * H + (half + 1) * 512],
                        start=True, stop=True, skip_group_check=True,
                    )
                    nc.vector.tensor_copy(
                        ab_t[b][:, half * 512:(half + 1) * 512], abp[:]
                    )

                # h1 eviction rotation: 11 of 16 on ACT, 5 on DVE (DVE also
                # owns every msc tensor_tensor eviction)
                ACT_SLOTS = {0, 1, 2, 4, 5, 7, 8, 10, 11, 13, 14}

                def relu_evict(dst, src):
                    i = ew_ctr[0] % 16
                    ew_ctr[0] += 1
                    if i in ACT_SLOTS:
                        nc.scalar.activation(dst, src, AF.Relu)
                    else:
                        nc.vector.tensor_scalar(dst, src, 0.0, None, ALU.max)

                def relu_bias(dst, src, bias):
                    # tail (output MLP) only
                    i = ew_ctr[0] % 16
                    ew_ctr[0] += 1
                    if i in ACT_SLOTS:
                        nc.scalar.activation(dst, src, AF.Relu, bias=bias)
                    else:
                        nc.vector.tensor_scalar(dst, src, bias, 0.0, ALU.add, ALU.max)

                def emit_step(k):
                    """Interleaved emission: h1(k), l2(k-1), scatter(k-2)."""
                    do_h1 = k < NK
                    do_l2 = 0 <= k - 1 < NK
                    do_sc = 0 <= k - 2 < NK

                    if do_sc:
                        bs, rs = divmod(k - 2, 4 * NST2)
                        ts, s2s = divmod(rs, NST2)
                        if rs == 0:
                            agg_t[bs] = apsum.tile([O, N], dtf, tag="agg", name="agg")
                        aggp = agg_t[bs]

                    def sc_mm(sub):
                        et = s2s * 8 + sub
                        col = (ts * NT + et) * 2
                        nc.tensor.matmul(
                            aggp[:, 2 * et:2 * et + 2],
                            msc_t[k - 2][:, sub * 64:(sub + 1) * 64],
                            rtm_t[bs][:, col:col + 2],
                            start=(rs == 0 and sub == 0), stop=(ts == 3),
                            skip_group_check=True,
                        )

                    if do_h1:
                        b, r = divmod(k, 4 * NST2)
                        t, s2 = divmod(r, NST2)
                        e0 = s2 * 1024
                        h1p = {}
                        h1s = {}
                        for c in range(2):
                            h1p[c] = hpsum.tile([128, 1024], dtf, tag="h1p", name="h1p")
                            h1s[c] = h1pool.tile([128, 1024], dtb, tag="h1s", name="h1s")
                    if do_l2:
                        bl, rl = divmod(k - 1, 4 * NST2)
                        tl, _ = divmod(rl, NST2)
                        mp = mpsum.tile([128, 512], dtf, tag="mp", name="mp")

                    def h1_mm(c, piece):
                        lw = ab_t[b][:, t * 256 + c * 128: t * 256 + (c + 1) * 128]
                        nc.tensor.matmul(
                            h1p[c][:, piece * 512:(piece + 1) * 512],
                            lw, srp[:, e0 + piece * 512:e0 + (piece + 1) * 512],
                            start=True, stop=True, skip_group_check=True,
                        )

                    def l2_mm(sub):
                        for c in range(2):
                            nc.tensor.matmul(
                                mp[:, sub * 64:(sub + 1) * 64],
                                h1_t[(k - 1, c)][:, sub * 128:(sub + 1) * 128],
                                w2s[:, (tl * 2 + c) * O:(tl * 2 + c + 1) * O],
                                start=(c == 0), stop=(c == 1),
                                skip_group_check=True,
                            )

                    # --- interleaved PE stream: long h1 matmuls hide the
                    # stationary reloads of the 16 small l2 matmuls ---
                    if do_h1:
                        h1_mm(0, 0)
                    if do_l2:
                        l2_mm(0)
                        l2_mm(1)
                    if do_sc:
                        sc_mm(0)
                        sc_mm(1)
                    if do_h1:
                        h1_mm(0, 1)
                    if do_l2:
                        l2_mm(2)
                        l2_mm(3)
                    if do_sc:
                        sc_mm(2)
                        sc_mm(3)
                    if do_h1:
                        relu_evict(h1s[0][:], h1p[0][:])
                        h1_mm(1, 0)
                    if do_l2:
                        l2_mm(4)
                        l2_mm(5)
                    if do_sc:
                        sc_mm(4)
                        sc_mm(5)
                    if do_h1:
                        h1_mm(1, 1)
                    if do_l2:
                        l2_mm(6)
                        l2_mm(7)
                    if do_sc:
                        sc_mm(6)
                        sc_mm(7)
                    if do_h1:
                        relu_evict(h1s[1][:], h1p[1][:])
                        h1_t[(k, 0)] = h1s[0]
                        h1_t[(k, 1)] = h1s[1]
                    if do_l2:
                        msc = mscpool.tile([128, 512], dtb, tag="msc", name="msc")
                        nc.vector.tensor_tensor(
                            msc[:], mp[:], nb2[:, tl * 512:(tl + 1) * 512], ALU.max
                        )
                        msc_t[k - 1] = msc

                    if do_sc:
                        if rs == 4 * NST2 - 1:
                            # batch done: assemble aug^T column block (x on top,
                            # agg + host-side b2*rowsum(rel_type) correction
                            # below) for the batched output MLP
                            nc.gpsimd.tensor_copy(
                                augT[0:64, bs * N:(bs + 1) * N], xt_t[bs][0:64, :]
                            )
                            nc.vector.tensor_tensor(
                                augT[64:128, bs * N:(bs + 1) * N],
                                aggp[:], corr_t[bs][:], ALU.add,
                            )

            # prologue (once per exec): batch 0/1 inputs + batch-0 A/B tiles
            emit_dma(0)
            emit_dma(1)
            emit_ab(0, 0)
            emit_ab(0, 1)

            loop_cm = (tc.For_i(0, reps, 1, staggered_reset=True)
                       if reps > 1 else contextlib.nullcontext())
            with loop_cm:
                # --- software-pipelined main loop over (batch, type, st2);
                # DMA/AB for the next rep are emitted at the tail (wraparound)
                # so each rep starts with its first AB tile already computed ---
                for k in range(NK + 2):
                    if k < NK:
                        b, r = divmod(k, 4 * NST2)
                        if r == 4:
                            emit_dma((b + 2) % bpc)
                        if r == 4 * NST2 - 3:
                            emit_ab((b + 1) % bpc, 0)
                        if r == 4 * NST2 - 2:
                            emit_ab((b + 1) % bpc, 1)
                    emit_step(k)

                # --- batched output MLP on aug^T [128, bpc*N] ---
                BN = bpc * N
                f1s = h1pool.tile([128, 2 * BN], dtb, tag="f1s")
                for mc in range(2):
                    fp = hpsum.tile([128, BN], dtf, tag="h1p", name="fp")
                    nc.tensor.matmul(
                        fp[:], ow1s[:, mc * 128:(mc + 1) * 128], augT[:],
                        start=True, stop=True,
                    )
                    relu_bias(f1s[:, mc * BN:(mc + 1) * BN], fp[:],
                              ob1c[:, mc:mc + 1])
                f2s = h1pool.tile([128, 2 * BN], dtb, tag="f2s")
                for mc in range(2):
                    fp = hpsum.tile([128, BN], dtf, tag="h1p", name="fp")
                    for kc in range(2):
                        nc.tensor.matmul(
                            fp[:], ow2s[:, (kc * 2 + mc) * 128:(kc * 2 + mc + 1) * 128],
                            f1s[:, kc * BN:(kc + 1) * BN],
                            start=(kc == 0), stop=(kc == 1),
                        )
                    relu_bias(f2s[:, mc * BN:(mc + 1) * BN], fp[:],
                              ob2c[:, mc:mc + 1])
                yp = mpsum.tile([O, BN], dtf, tag="mp", name="yp")
                for kc in range(2):
                    nc.tensor.matmul(
                        yp[:], ow3s[:, kc * O:(kc + 1) * O],
                        f2s[:, kc * BN:(kc + 1) * BN],
                        start=(kc == 0), stop=(kc == 1),
                    )
                y_sb = wpool.tile([O, BN], dtf, tag="ysb")
                nc.vector.tensor_scalar(
                    y_sb[:], yp[:], ob3c[:, 0:1], None, ALU.add
                )
                nc.sync.dma_start(y_d[:], y_sb[:])

    nc.compile()
    return nc


def prep_shared(rel_rec, rel_send, w1, b1, w2, b2, ow1, ob1, ow2, ob2, ow3, ob3):
    """Host-side layout prep for the replicated tensors."""
    f32 = np.float32
    srp = np.zeros((128, EP), f32)
    ee = np.arange(EP)
    srp[ee % 64, ee] = 1.0          # sender one-hot rows (A, i = e'%64)
    srp[64 + ee // 64, ee] = 1.0    # receiver one-hot rows (B, j = e'//64)
    # w1 with b1/2 folded in as a 65th contraction row (pairs with the
    # ones-row appended to xT)
    w1top = np.concatenate([w1[:, :F, :], 0.5 * b1[:, None, :]], axis=1)
    w1bot = np.concatenate([w1[:, F:, :], 0.5 * b1[:, None, :]], axis=1)
    w1s = np.concatenate([
        w1top.transpose(1, 0, 2).reshape(F + 1, T * H),
        w1bot.transpose(1, 0, 2).reshape(F + 1, T * H),
    ], axis=1)
    w2s = w2.reshape(T, 2, 128, O).transpose(2, 0, 1, 3).reshape(128, T * 2 * O)
    # resident -b2 tile: [128 partitions] x [t, sub(8), o]
    nb2 = np.tile(-b2.reshape(T, 1, 1, O), (1, 128, 8, 1))
    nb2 = nb2.transpose(1, 0, 2, 3).reshape(128, T * 512)
    ow2s = ow2.reshape(2, 128, 2, 128).transpose(1, 0, 2, 3).reshape(128, 512)
    ow3s = ow3.reshape(2, 128, O).transpose(1, 0, 2).reshape(128, 2 * O)
    bf = lambda a: np.ascontiguousarray(a).astype(BF16)
    return dict(
        srp=bf(srp), w1s=bf(w1s), w2s=bf(w2s), nb2=bf(nb2),
        ow1s=bf(ow1), ow2s=bf(ow2s), ow3s=bf(ow3s),
        ob1c=np.ascontiguousarray(ob1.reshape(2, 128).T, f32),
        ob2c=np.ascontiguousarray(ob2.reshape(2, 128).T, f32),
        ob3c=np.ascontiguousarray(ob3.reshape(O, 1), f32),
    )


def prep_batch(x, rel_type, b2):
    """Per-core batched tensors: xT (+ones row), packed rel_type, b2 corr."""
    f32 = np.float32
    bpc = x.shape[0]
    xT = np.concatenate(
        [x.transpose(0, 2, 1), np.ones((bpc, 1, N), f32)], axis=1
    ).astype(BF16)
    # reference edge order: i-major over (i, j), i != j
    ii, jj = np.where(~np.eye(N, dtype=bool))
    rtg = np.zeros((bpc, N, N, T), f32)
    rtg[:, ii, jj, :] = rel_type            # [b, i, j, t], zero diagonal
    rtp = rtg.transpose(0, 2, 1, 3)         # [b, j, i, t] receiver-major
    rtm = np.zeros((bpc, 128, T, NT, 2), f32)
    rtm[:, 0:64, :, :, 0] = rtp[:, 0::2].transpose(0, 2, 3, 1)
    rtm[:, 64:128, :, :, 1] = rtp[:, 1::2].transpose(0, 2, 3, 1)
    rtm = rtm.reshape(bpc, 128, T * NT * 2).astype(BF16)
    # +b2 compensation for the max(z, -b2) eviction:
    #   corr[b, o, j] = sum_t b2[t, o] * sum_i rel_type[b, (i->j), t]
    S = rtg.sum(axis=1)                     # [b, j, t]
    corr = np.einsum("bjt,to->boj", S, b2.astype(f32)).astype(f32)
    return {"xT": xT, "rtm": rtm, "corr": np.ascontiguousarray(corr)}


def kernel(**inputs):
    from concourse.bass_utils import run_bass_kernel_spmd

    f32arrs = {k: np.asarray(v, dtype=np.float32) for k, v in inputs.items()}
    shared = prep_shared(
        f32arrs["rel_rec"], f32arrs["rel_send"],
        f32arrs["w1"], f32arrs["b1"], f32arrs["w2"], f32arrs["b2"],
        f32arrs["ow1"], f32arrs["ob1"], f32arrs["ow2"], f32arrs["ob2"],
        f32arrs["ow3"], f32arrs["ob3"],
    )
    in_maps = []
    for c in range(NCORES):
        sl = slice(c * BPC, (c + 1) * BPC)
        m = dict(shared)
        m.update(prep_batch(f32arrs["x"][sl], f32arrs["rel_type"][sl],
                            f32arrs["b2"]))
        in_maps.append(m)

    nc = build_nc(BPC)
    res = run_bass_kernel_spmd(nc, in_maps, list(range(NCORES)))
    # y per core: [O, BPC*N] -> [BPC, N, O]
    y = np.concatenate(
        [res.results[c]["y"].reshape(O, BPC, N).transpose(1, 2, 0)
         for c in range(NCORES)], axis=0
    )
    return np.ascontiguousarray(y).astype(np.float32)


# revision 3
# speedup vs baseline: 1.0465x; 1.0465x over previous
"""Trainium2 Bass kernel v2 for NRI-style GNN decoder (nn_Decoder_58600533787128).

Data-parallel over batch across 8 NeuronCores, bf16 matmul datapath.

v2 changes over v1:
  - b1 folded into the A/B node-feature GEMM via a ones-row (K=64 -> 65);
    h1 eviction is a pure relu (no bias operand).
  - b2 broadcast matmul eliminated: layer-2 eviction computes
    max(z, -b2) (DVE tensor_tensor against a resident -b2 tile) and the
    +b2 compensation  sum_t b2_t (x) rowsum_i(rel_type_t)  is computed on
    host per batch and added (fp32) when agg PSUM is copied into augT.
  - PSUM repack: ab shares the mp pool, agg double-buffered.
  - layer-2 matmuls interleaved between the long h1 matmuls (LDW hiding).
  - ACT/DVE eviction rotation rebalanced (msc eviction is DVE-only now).
"""
import sys

sys.path.insert(0, "/opt/trn_rl_repo")

import numpy as np
import ml_dtypes

BF16 = ml_dtypes.bfloat16

B, N, F, H, O, T, E = 64, 64, 64, 256, 64, 4, 4032
EP = 4096           # padded edge count (full N*N grid, diagonal dead)
NT = EP // 128      # 32 edge tiles of 128 (2 receiver blocks each)
NCORES = 8
BPC = B // NCORES   # batches per core
NST2 = 4            # 4 supertiles of 1024 edges per batch


def build_nc(bpc=BPC, num_devices=NCORES, reps=1):
    import concourse.mybir as mybir
    from concourse import bacc, tile

    dtf = mybir.dt.float32
    dtb = mybir.dt.bfloat16
    AF = mybir.ActivationFunctionType
    ALU = mybir.AluOpType

    nc = bacc.Bacc(
        "TRN2", target_bir_lowering=False, debug=False, num_devices=num_devices
    )
    xT_d = nc.declare_dram_parameter("xT", [bpc, F + 1, N], dtb, isOutput=False)
    rtm_d = nc.declare_dram_parameter("rtm", [bpc, 128, T * NT * 2], dtb, isOutput=False)
    corr_d = nc.declare_dram_parameter("corr", [bpc, O, N], dtf, isOutput=False)
    srp_d = nc.declare_dram_parameter("srp", [128, EP], dtb, isOutput=False)
    w1_d = nc.declare_dram_parameter("w1s", [F + 1, 2 * T * H], dtb, isOutput=False)
    w2_d = nc.declare_dram_parameter("w2s", [128, T * 2 * O], dtb, isOutput=False)
    nb2_d = nc.declare_dram_parameter("nb2", [128, T * 512], dtb, isOutput=False)
    ow1_d = nc.declare_dram_parameter("ow1s", [128, H], dtb, isOutput=False)
    ow2_d = nc.declare_dram_parameter("ow2s", [128, 4 * 128], dtb, isOutput=False)
    ow3_d = nc.declare_dram_parameter("ow3s", [128, 2 * O], dtb, isOutput=False)
    ob1_d = nc.declare_dram_parameter("ob1c", [128, 2], dtf, isOutput=False)
    ob2_d = nc.declare_dram_parameter("ob2c", [128, 2], dtf, isOutput=False)
    ob3_d = nc.declare_dram_parameter("ob3c", [O, 1], dtf, isOutput=False)
    y_d = nc.declare_dram_parameter("y", [O, bpc * N], dtf, isOutput=True)

    NK = bpc * 4 * NST2  # pipeline stages: (batch, type, st2)

    with tile.TileContext(nc) as tc:
        with (
            tc.tile_pool(name="const", bufs=1) as cpool,
            tc.tile_pool(name="work", bufs=4) as wpool,
            tc.tile_pool(name="absb", bufs=2) as abpool_sb,
            tc.tile_pool(name="h1sb", bufs=6) as h1pool,
            tc.tile_pool(name="mscsb", bufs=6) as mscpool,
            tc.tile_pool(name="hps", bufs=2, space="PSUM") as hpsum,
            tc.tile_pool(name="mps", bufs=2, space="PSUM") as mpsum,
            tc.tile_pool(name="aggps", bufs=2, space="PSUM") as apsum,
        ):
            # resident constants (one DMA each; layouts prepped host-side)
            srp = cpool.tile([128, EP], dtb)
            nc.sync.dma_start(srp[:], srp_d[:])
            w1s = cpool.tile([F + 1, 2 * T * H], dtb)
            nc.sync.dma_start(w1s[:], w1_d[:])
            w2s = cpool.tile([128, T * 2 * O], dtb)
            nc.sync.dma_start(w2s[:], w2_d[:])
            nb2 = cpool.tile([128, T * 512], dtb)
            nc.sync.dma_start(nb2[:], nb2_d[:])
            ow1s = cpool.tile([128, H], dtb)
            nc.sync.dma_start(ow1s[:], ow1_d[:])
            ow2s = cpool.tile([128, 4 * 128], dtb)
            nc.sync.dma_start(ow2s[:], ow2_d[:])
            ow3s = cpool.tile([128, 2 * O], dtb)
            nc.sync.dma_start(ow3s[:], ow3_d[:])
            ob1c = cpool.tile([128, 2], dtf)
            nc.sync.dma_start(ob1c[:], ob1_d[:])
            ob2c = cpool.tile([128, 2], dtf)
            nc.sync.dma_start(ob2c[:], ob2_d[:])
            ob3c = cpool.tile([O, 1], dtf)
            nc.sync.dma_start(ob3c[:], ob3_d[:])

            import contextlib
            augT = wpool.tile([128, bpc * N], dtb, tag="augT")
            if True:
                xt_t = {}
                rtm_t = {}
                corr_t = {}
                ab_t = {}
                h1_t = {}    # (k, c) -> sbuf tile [128, 1024]
                msc_t = {}   # k -> sbuf tile [128, 512]
                agg_t = {}   # b -> psum tile [64, 64]
                ew_ctr = [0]

                def emit_dma(b):
                    xt_t[b] = wpool.tile([F + 1, N], dtb, tag="xt", name="xt")
                    nc.sync.dma_start(xt_t[b][:], xT_d[b])
                    rtm_t[b] = wpool.tile([128, T * NT * 2], dtb, tag="rtm", name="rtm")
                    nc.sync.dma_start(rtm_t[b][:], rtm_d[b])
                    corr_t[b] = wpool.tile([O, N], dtf, tag="corr", name="corr")
                    nc.sync.dma_start(corr_t[b][:], corr_d[b])

                def emit_ab(b, half):
                    # A/B node features for 2 edge types: [A_t|A_t+1 ; B_t|B_t+1]
                    # (b1/2 folded in via the ones-row of xT')
                    if half == 0:
                        ab_t[b] = abpool_sb.tile([128, T * 256], dtb, tag="abs", name="abs")
                    abp = mpsum.tile([128, 512], dtf, tag="mp", name="abp")
                    nc.tensor.matmul(
                        abp[0:64, :], xt_t[b][:], w1s[:, half * 512:(half + 1) * 512],
                        start=True, stop=True, skip_group_check=True,
                    )
                    nc.tensor.matmul(
                        abp[64:128, :], xt_t[b][:],
                        w1s[:, T * H + half * 512: T * H + (half + 1) * 512],
                        start=True, stop=True, skip_group_check=True,
                    )
                    nc.vector.tensor_copy(
                        ab_t[b][:, half * 512:(half + 1) * 512], abp[:]
                    )

                # h1 eviction rotation: 11 of 16 on ACT, 5 on DVE (DVE also
                # owns every msc tensor_tensor eviction)
                ACT_SLOTS = {0, 1, 2, 4, 5, 7, 8, 10, 11, 13, 14}

                def relu_evict(dst, src):
                    i = ew_ctr[0] % 16
                    ew_ctr[0] += 1
                    if i in ACT_SLOTS:
                        nc.scalar.activation(dst, src, AF.Relu)
                    else:
                        nc.vector.tensor_scalar(dst, src, 0.0, None, ALU.max)

                def relu_bias(dst, src, bias):
                    # tail (output MLP) only
                    i = ew_ctr[0] % 16
                    ew_ctr[0] += 1
                    if i in ACT_SLOTS:
                        nc.scalar.activation(dst, src, AF.Relu, bias=bias)
                    else:
                        nc.vector.tensor_scalar(dst, src, bias, 0.0, ALU.add, ALU.max)

                def emit_step(k):
                    """Interleaved emission: h1(k), l2(k-1), scatter(k-2)."""
                    do_h1 = k < NK
                    do_l2 = 0 <= k - 1 < NK
                    do_sc = 0 <= k - 2 < NK

                    if do_h1:
                        b, r = divmod(k, 4 * NST2)
                        t, s2 = divmod(r, NST2)
                        e0 = s2 * 1024
                        h1p = {}
                        h1s = {}
                        for c in range(2):
                            h1p[c] = hpsum.tile([128, 1024], dtf, tag="h1p", name="h1p")
                            h1s[c] = h1pool.tile([128, 1024], dtb, tag="h1s", name="h1s")
                    if do_l2:
                        bl, rl = divmod(k - 1, 4 * NST2)
                        tl, _ = divmod(rl, NST2)
                        mp = mpsum.tile([128, 512], dtf, tag="mp", name="mp")

                    def h1_mm(c, piece):
                        lw = ab_t[b][:, t * 256 + c * 128: t * 256 + (c + 1) * 128]
                        nc.tensor.matmul(
                            h1p[c][:, piece * 512:(piece + 1) * 512],
                            lw, srp[:, e0 + piece * 512:e0 + (piece + 1) * 512],
                            start=True, stop=True, skip_group_check=True,
                        )

                    def l2_mm(sub):
                        for c in range(2):
                            nc.tensor.matmul(
                                mp[:, sub * 64:(sub + 1) * 64],
                                h1_t[(k - 1, c)][:, sub * 128:(sub + 1) * 128],
                                w2s[:, (tl * 2 + c) * O:(tl * 2 + c + 1) * O],
                                start=(c == 0), stop=(c == 1),
                                skip_group_check=True,
                            )

                    # --- interleaved PE stream: long h1 matmuls hide the
                    # stationary reloads of the 16 small l2 matmuls ---
                    if do_h1:
                        h1_mm(0, 0)
                    if do_l2:
                        l2_mm(0)
                        l2_mm(1)
                    if do_h1:
                        h1_mm(0, 1)
                    if do_l2:
                        l2_mm(2)
                        l2_mm(3)
                    if do_h1:
                        relu_evict(h1s[0][:], h1p[0][:])
                        h1_mm(1, 0)
                    if do_l2:
                        l2_mm(4)
                        l2_mm(5)
                    if do_h1:
                        h1_mm(1, 1)
                    if do_l2:
                        l2_mm(6)
                        l2_mm(7)
                    if do_h1:
                        relu_evict(h1s[1][:], h1p[1][:])
                        h1_t[(k, 0)] = h1s[0]
                        h1_t[(k, 1)] = h1s[1]
                    if do_l2:
                        msc = mscpool.tile([128, 512], dtb, tag="msc", name="msc")
                        nc.vector.tensor_tensor(
                            msc[:], mp[:], nb2[:, tl * 512:(tl + 1) * 512], ALU.max
                        )
                        msc_t[k - 1] = msc

                    if do_sc:
                        bs, rs = divmod(k - 2, 4 * NST2)
                        ts, s2s = divmod(rs, NST2)
                        if rs == 0:
                            agg_t[bs] = apsum.tile([O, N], dtf, tag="agg", name="agg")
                        aggp = agg_t[bs]
                        for sub in range(8):
                            et = s2s * 8 + sub
                            col = (ts * NT + et) * 2
                            nc.tensor.matmul(
                                aggp[:, 2 * et:2 * et + 2],
                                msc_t[k - 2][:, sub * 64:(sub + 1) * 64],
                                rtm_t[bs][:, col:col + 2],
                                start=(rs == 0 and sub == 0), stop=(ts == 3),
                                skip_group_check=True,
                            )
                        if rs == 4 * NST2 - 1:
                            # batch done: assemble aug^T column block (x on top,
                            # agg + host-side b2*rowsum(rel_type) correction
                            # below) for the batched output MLP
                            nc.gpsimd.tensor_copy(
                                augT[0:64, bs * N:(bs + 1) * N], xt_t[bs][0:64, :]
                            )
                            nc.vector.tensor_tensor(
                                augT[64:128, bs * N:(bs + 1) * N],
                                aggp[:], corr_t[bs][:], ALU.add,
                            )

            # prologue (once per exec): batch 0/1 inputs + batch-0 A/B tiles
            emit_dma(0)
            emit_dma(1)
            emit_ab(0, 0)
            emit_ab(0, 1)

            loop_cm = (tc.For_i(0, reps, 1, staggered_reset=True)
                       if reps > 1 else contextlib.nullcontext())
            with loop_cm:
                # --- software-pipelined main loop over (batch, type, st2);
                # DMA/AB for the next rep are emitted at the tail (wraparound)
                # so each rep starts with its first AB tile already computed ---
                for k in range(NK + 2):
                    if k < NK:
                        b, r = divmod(k, 4 * NST2)
                        if r == 4:
                            emit_dma((b + 2) % bpc)
                        if r == 4 * NST2 - 3:
                            emit_ab((b + 1) % bpc, 0)
                        if r == 4 * NST2 - 2:
                            emit_ab((b + 1) % bpc, 1)
                    emit_step(k)

                # --- batched output MLP on aug^T [128, bpc*N] ---
                BN = bpc * N
                f1s = h1pool.tile([128, 2 * BN], dtb, tag="f1s")
                for mc in range(2):
                    fp = hpsum.tile([128, BN], dtf, tag="h1p", name="fp")
                    nc.tensor.matmul(
                        fp[:], ow1s[:, mc * 128:(mc + 1) * 128], augT[:],
                        start=True, stop=True,
                    )
                    relu_bias(f1s[:, mc * BN:(mc + 1) * BN], fp[:],
                              ob1c[:, mc:mc + 1])
                f2s = h1pool.tile([128, 2 * BN], dtb, tag="f2s")
                for mc in range(2):
                    fp = hpsum.tile([128, BN], dtf, tag="h1p", name="fp")
                    for kc in range(2):
                        nc.tensor.matmul(
                            fp[:], ow2s[:, (kc * 2 + mc) * 128:(kc * 2 + mc + 1) * 128],
                            f1s[:, kc * BN:(kc + 1) * BN],
                            start=(kc == 0), stop=(kc == 1),
                        )
                    relu_bias(f2s[:, mc * BN:(mc + 1) * BN], fp[:],
                              ob2c[:, mc:mc + 1])
                yp = mpsum.tile([O, BN], dtf, tag="mp", name="yp")
                for kc in range(2):
                    nc.tensor.matmul(
                        yp[:], ow3s[:, kc * O:(kc + 1) * O],
                        f2s[:, kc * BN:(kc + 1) * BN],
                        start=(kc == 0), stop=(kc == 1),
                    )
                y_sb = wpool.tile([O, BN], dtf, tag="ysb")
                nc.vector.tensor_scalar(
                    y_sb[:], yp[:], ob3c[:, 0:1], None, ALU.add
                )
                nc.sync.dma_start(y_d[:], y_sb[:])

    nc.compile()
    return nc


def prep_shared(rel_rec, rel_send, w1, b1, w2, b2, ow1, ob1, ow2, ob2, ow3, ob3):
    """Host-side layout prep for the replicated tensors."""
    f32 = np.float32
    srp = np.zeros((128, EP), f32)
    ee = np.arange(EP)
    srp[ee % 64, ee] = 1.0          # sender one-hot rows (A, i = e'%64)
    srp[64 + ee // 64, ee] = 1.0    # receiver one-hot rows (B, j = e'//64)
    # w1 with b1/2 folded in as a 65th contraction row (pairs with the
    # ones-row appended to xT)
    w1top = np.concatenate([w1[:, :F, :], 0.5 * b1[:, None, :]], axis=1)
    w1bot = np.concatenate([w1[:, F:, :], 0.5 * b1[:, None, :]], axis=1)
    w1s = np.concatenate([
        w1top.transpose(1, 0, 2).reshape(F + 1, T * H),
        w1bot.transpose(1, 0, 2).reshape(F + 1, T * H),
    ], axis=1)
    w2s = w2.reshape(T, 2, 128, O).transpose(2, 0, 1, 3).reshape(128, T * 2 * O)
    # resident -b2 tile: [128 partitions] x [t, sub(8), o]
    nb2 = np.tile(-b2.reshape(T, 1, 1, O), (1, 128, 8, 1))
    nb2 = nb2.transpose(1, 0, 2, 3).reshape(128, T * 512)
    ow2s = ow2.reshape(2, 128, 2, 128).transpose(1, 0, 2, 3).reshape(128, 512)
    ow3s = ow3.reshape(2, 128, O).transpose(1, 0, 2).reshape(128, 2 * O)
    bf = lambda a: np.ascontiguousarray(a).astype(BF16)
    return dict(
        srp=bf(srp), w1s=bf(w1s), w2s=bf(w2s), nb2=bf(nb2),
        ow1s=bf(ow1), ow2s=bf(ow2s), ow3s=bf(ow3s),
        ob1c=np.ascontiguousarray(ob1.reshape(2, 128).T, f32),
        ob2c=np.ascontiguousarray(ob2.reshape(2, 128).T, f32),
        ob3c=np.ascontiguousarray(ob3.reshape(O, 1), f32),
    )


def prep_batch(x, rel_type, b2):
    """Per-core batched tensors: xT (+ones row), packed rel_type, b2 corr."""
    f32 = np.float32
    bpc = x.shape[0]
    xT = np.concatenate(
        [x.transpose(0, 2, 1), np.ones((bpc, 1, N), f32)], axis=1
    ).astype(BF16)
    # reference edge order: i-major over (i, j), i != j
    ii, jj = np.where(~np.eye(N, dtype=bool))
    rtg = np.zeros((bpc, N, N, T), f32)
    rtg[:, ii, jj, :] = rel_type            # [b, i, j, t], zero diagonal
    rtp = rtg.transpose(0, 2, 1, 3)         # [b, j, i, t] receiver-major
    rtm = np.zeros((bpc, 128, T, NT, 2), f32)
    rtm[:, 0:64, :, :, 0] = rtp[:, 0::2].transpose(0, 2, 3, 1)
    rtm[:, 64:128, :, :, 1] = rtp[:, 1::2].transpose(0, 2, 3, 1)
    rtm = rtm.reshape(bpc, 128, T * NT * 2).astype(BF16)
    # +b2 compensation for the max(z, -b2) eviction:
    #   corr[b, o, j] = sum_t b2[t, o] * sum_i rel_type[b, (i->j), t]
    S = rtg.sum(axis=1)                     # [b, j, t]
    corr = np.einsum("bjt,to->boj", S, b2.astype(f32)).astype(f32)
    return {"xT": xT, "rtm": rtm, "corr": np.ascontiguousarray(corr)}


def kernel(**inputs):
    from concourse.bass_utils import run_bass_kernel_spmd

    f32arrs = {k: np.asarray(v, dtype=np.float32) for k, v in inputs.items()}
    shared = prep_shared(
        f32arrs["rel_rec"], f32arrs["rel_send"],
        f32arrs["w1"], f32arrs["b1"], f32arrs["w2"], f32arrs["b2"],
        f32arrs["ow1"], f32arrs["ob1"], f32arrs["ow2"], f32arrs["ob2"],
        f32arrs["ow3"], f32arrs["ob3"],
    )
    in_maps = []
    for c in range(NCORES):
        sl = slice(c * BPC, (c + 1) * BPC)
        m = dict(shared)
        m.update(prep_batch(f32arrs["x"][sl], f32arrs["rel_type"][sl],
                            f32arrs["b2"]))
        in_maps.append(m)

    nc = build_nc(BPC)
    res = run_bass_kernel_spmd(nc, in_maps, list(range(NCORES)))
    # y per core: [O, BPC*N] -> [BPC, N, O]
    y = np.concatenate(
        [res.results[c]["y"].reshape(O, BPC, N).transpose(1, 2, 0)
         for c in range(NCORES)], axis=0
    )
    return np.ascontiguousarray(y).astype(np.float32)
